# revision 9
# baseline (speedup 1.0000x reference)
"""Trainium2 Bass kernel: 8-connectivity CCL of a 4096x4096 binary image
(prob > 0.5); labels = min linear index in component + 1, background 0.

Single-launch, transpose-free design: image split into 8 row-strips of 512
rows, one per NeuronCore. Each core solves its strip to a LOCAL fixpoint on
device with a 6-level multigrid min-propagation scheme, all arrays kept in
row-major form; vertical data movement (3x3 sweep, 2x2 restriction,
prolongation, gated +-1 vertical steps, nef-gate assembly) is done with
partition-shifted / partition-strided SBUF-to-SBUF DMAs, so only the DMA
and DVE(vector) engines are used. The L1 block-edge gates are built on
device from the foreground mask. NCYC unrolled V-cycles; each cycle starts
with an exact Jacobi 3x3 masked min sweep, so "last cycle changed nothing"
(checked via per-cycle change counts) certifies strip-exact labels. Host
then merges the 7 seam equivalences (tiny union-find) and remaps.

Input per core: bit-packed foreground u8 [512, 512] (np.packbits of
prob > 0.5, bit 1 = foreground). Output per core: flat int32
[1, 512*4096 + 128*NCYC]: labels row-major (0 = bg, strip-local values),
then per-cycle change counts [128, NCYC].
"""
import sys
sys.path.insert(0, '/opt/trn_rl_repo')
sys.path.insert(0, '/root/.axon_site')
sys.path.insert(0, '/root/.axon_site/_ro/trn_rl_repo')
import numpy as np
from contextlib import ExitStack

import concourse.bass as bass
import concourse.bacc as bacc
import concourse.mybir as mybir
import concourse.tile as tile
from concourse.bass_utils import run_bass_kernel_spmd

F32 = mybir.dt.float32
I32 = mybir.dt.int32
U8 = mybir.dt.uint8
BF16 = mybir.dt.bfloat16
U16 = mybir.dt.uint16
AL = mybir.AluOpType
AX = mybir.AxisListType

H = W = 4096
NCORES = 8
SR = H // NCORES            # 512
YT = SR // 128              # 4
WP = W // 8                 # packed bytes per row
BIG = float(2 ** 25)
BIGI = np.int64(2 ** 25)
NCYC = 13
NS0 = 2
NS1 = 2
NLEV = 6
LEV = {k: (SR >> k, W >> k) for k in range(NLEV)}


def dbl(ap):
    """stride-0 double the last free dim: [p, n] -> reads each elem twice"""
    return ap.unsqueeze(2).broadcast_to([ap.shape[0], ap.shape[1], 2])


class Dev:
    def __init__(self, tc, ins, outs, ncyc):
        self.tc = tc
        self.nc = tc.nc
        self.ins = ins
        self.outs = outs
        self.ncyc = ncyc

    def S(self):
        return self.scr.tile([128, W], F32, tag="S", name="S")

    def S2(self):
        return self.scr.tile([128, W], F32, tag="S2", name="S2")

    def build(self):
        nc, tc = self.nc, self.tc
        ctx = ExitStack()
        with ctx:
            pers = ctx.enter_context(tc.tile_pool(name="pers", bufs=1))
            self.scr = ctx.enter_context(tc.tile_pool(name="scr", bufs=1))
            dpool = ctx.enter_context(
                tc.tile_pool(name="dscratch", bufs=1, space="DRAM"))

            R0 = [pers.tile([128, W], F32, tag=f"R0_{b}", name=f"R0_{b}")
                  for b in range(YT)]
            GBb = [pers.tile([128, W], BF16, tag=f"GB_{b}", name=f"GB_{b}")
                   for b in range(YT)]
            r1, w1 = LEV[1]
            gh1s = pers.tile([128, 2 * w1], BF16, tag="gh1f", name="gh1f")
            gv1s = pers.tile([128, 2 * w1], BF16, tag="gv1f", name="gv1f")
            gh1f = [gh1s[:, i * w1:(i + 1) * w1] for i in range(2)]
            gv1f = [gv1s[:, i * w1:(i + 1) * w1] for i in range(2)]
            L1R = [pers.tile([128, w1], F32, tag=f"L1R_{i}", name=f"L1R_{i}")
                   for i in range(2)]
            chga = pers.tile([128, self.ncyc], F32, tag="chga", name="chga")
            nc.vector.memset(chga[:], 0.0)
            self.R0, self.GBb, self.L1R = R0, GBb, L1R
            self.gh1f, self.gv1f = gh1f, gv1f

            self.lab = {1: L1R}
            self.Hf = {}
            self.Vf = {}
            self.snapM = {}
            for k in range(2, NLEV):
                rk, wk = LEV[k]
                self.lab[k] = [pers.tile([rk, wk], F32, tag=f"lab{k}",
                                         name=f"lab{k}")]
                self.Hf[k] = [pers.tile([rk, wk], F32, tag=f"Hf{k}",
                                        name=f"Hf{k}")]
                self.Vf[k] = [pers.tile([rk, wk], F32, tag=f"Vf{k}",
                                        name=f"Vf{k}")]
                self.snapM[k] = pers.tile([rk, wk], F32, tag=f"snapM{k}",
                                          name=f"snapM{k}")

            self.prevt = [dpool.tile([128, W], F32, tag=f"prevd_{b}",
                                     name=f"prevd_{b}") for b in range(YT)]
            self.bigrow = dpool.tile([1, W], F32, tag="bigrowd",
                                     name="bigrowd")
            self.rowsnap = dpool.tile([8, W], F32, tag="rowsnap",
                                      name="rowsnap")
            zrow8 = dpool.tile([1, W], U8, tag="zrow8", name="zrow8")

            blob_r = self.ins["blob"].rearrange("(a p) w -> a p w", p=128)

            # ---- prologue: constants ----
            bs = self.S()
            nc.vector.memset(bs[0:1, :], BIG)
            nc.sync.dma_start(self.bigrow[:], bs[0:1, :])
            mk = self.scr.tile([128, 1], F32, tag="red", name="mk")
            mkv = mk.bitcast(I32)[:, 0:1]
            nc.gpsimd.iota(mkv, pattern=[[1, 1]], base=0, channel_multiplier=1)
            nc.vector.tensor_scalar(mkv, mkv, 1.0, None, op0=AL.bitwise_and)
            emask = pers.tile([128, 1], F32, tag="emask", name="emask")
            nc.vector.tensor_scalar(emask[:], mkv, -1.0, 1.0, op0=AL.mult,
                                    op1=AL.add)

            # ---- prologue: unpack fg + build gates (device-side) ----
            self._build_gates(blob_r, zrow8, emask)

            # ---- prologue: init labels ----
            for b in range(YT):
                ti = self.S()
                tiv = ti.bitcast(I32)[:, :W]
                nc.gpsimd.iota(tiv, pattern=[[1, W]], base=b * 128 * W + 1,
                               channel_multiplier=W)
                nc.vector.tensor_copy(R0[b][:], tiv)
                t1 = self.S2()
                nc.vector.tensor_scalar(t1[:], R0[b][:], -1.0, BIG,
                                        op0=AL.mult, op1=AL.add)
                nc.vector.scalar_tensor_tensor(t1[:], GBb[b][:], 1.0 / BIG,
                                               t1[:], op0=AL.mult,
                                               op1=AL.mult)
                nc.vector.tensor_tensor(R0[b][:], R0[b][:], t1[:], op=AL.add)
                nc.sync.dma_start(self.prevt[b][:], R0[b][:])

            # ---- cycles ----
            for c in range(self.ncyc):
                self.cycle(c, chga)

            # ---- epilogue: pack labels as u16 low + u8 high planes.
            # i32 bitwise ops (same-dtype), then strided-byte DMA extracts
            # the u16/u8 lanes directly into the output. ----
            SW = SR * W
            for b in range(YT):
                ne = self.S()
                nev = ne.bitcast(F32)[:, :W]
                nc.vector.tensor_scalar(nev, R0[b][:], BIG, 0.0,
                                        op0=AL.is_lt, op1=AL.add)
                oi = self.S2()
                oiv = oi.bitcast(I32)[:, :W]
                nc.vector.tensor_tensor(oiv, R0[b][:], nev, op=AL.mult)
                # low 16 bits, contiguous u16 lane extract via stride-2 view
                d16 = self.outs["out"][0:1, b * 128 * 2 * W:
                                       (b + 1) * 128 * 2 * W]
                d16r = d16.rearrange("o (p w) -> (o p) w",
                                     w=2 * W).bitcast(U16)
                for hh in range(2):
                    nc.sync.dma_start(
                        d16r[:, hh * (W // 2):(hh + 1) * (W // 2)],
                        oi.bitcast(U16)[:, hh * W:hh * W + W:2])
                # high bits: lab >> 16 in place, then stride-4 u8 extract
                nc.vector.tensor_scalar(oiv, oiv, 16.0, None,
                                        op0=AL.logical_shift_right)
                d8 = self.outs["out"][0:1, 2 * SW + b * 128 * W:
                                      2 * SW + (b + 1) * 128 * W]
                d8r = d8.rearrange("o (p w) -> (o p) w", w=W)
                for hh in range(2):
                    nc.sync.dma_start(
                        d8r[:, hh * (W // 2):(hh + 1) * (W // 2)],
                        oi.bitcast(U8)[:, hh * 2 * W:hh * 2 * W + 2 * W:4])
            ci = self.S()
            civ = ci.bitcast(I32)[:, :W]
            nc.vector.tensor_copy(civ[:, 0:self.ncyc], chga[:])
            cdst = self.outs["out"][0:1, 3 * SW:3 * SW + 512 * self.ncyc]
            nc.sync.dma_start(
                cdst.rearrange("o (p j) -> (o p) j", j=4 * self.ncyc),
                ci.bitcast(U8)[:, 0:4 * self.ncyc])

    # ------------------------------------------------------------------
    def _build_gates(self, blob_r, zrow8, emask):
        """Unpack fg per tile pair and build L1 gates gh1f/gv1f on device.

        gh[I,J] (J>=1) = BIG*NOT(EH1[I,J-1]); gv[I,J] (I>=1) = BIG*NOT(
        EV1[I-1,J]); col0/row0 = BIG. EH1/EV1 per the reference quadrant
        formulas; EH1 staged in L1R (f32) then converted, same for EV1."""
        nc, tc = self.nc, self.tc
        r1, w1 = LEV[1]
        with tc.tile_pool(name="gates", bufs=1) as pp:
            def unpack(b):
                pk = pp.tile([128, WP], U8, tag="pk", name="pk", bufs=2)
                nc.sync.dma_start(pk[:], blob_r[b])
                g = pp.tile([128, W], U8, tag="g", name="g", bufs=2)
                for j in range(8):
                    nc.vector.tensor_scalar(g[:, j::8], pk[:], float(7 - j),
                                            1.0,
                                            op0=AL.logical_shift_right,
                                            op1=AL.bitwise_and)
                return g

            gz = pp.tile([128, W], U8, tag="g", name="gz", bufs=2)
            nc.vector.memset(gz[0:1, :], 0)
            nc.sync.dma_start(zrow8[:], gz[0:1, :])
            g_next = unpack(0)
            for b in range(YT):
                g = g_next
                nc.vector.tensor_scalar(self.GBb[b][:], g[:], -BIG, BIG,
                                        op0=AL.mult, op1=AL.add)
                g_next = unpack(b + 1) if b + 1 < YT else None
                gup = pp.tile([128, W], U8, tag="gup", name="gup")
                nc.sync.dma_start(gup[0:127, :], g[1:128, :])
                if g_next is not None:
                    nc.sync.dma_start(gup[127:128, :], g_next[0:1, :])
                else:
                    nc.sync.dma_start(gup[127:128, :], zrow8[:])
                # ---- EH1 terms ----
                S = self.S()
                S2 = self.S2()
                # EH0 = g & g>>x
                nc.vector.tensor_tensor(S[:, :W - 1], g[:, :W - 1], g[:, 1:],
                                        op=AL.mult)
                nc.vector.memset(S[:, W - 1:], 0.0)
                # ED1 = g & gup>>x
                nc.vector.tensor_tensor(S2[:, :W - 1], g[:, :W - 1],
                                        gup[:, 1:], op=AL.mult)
                nc.vector.memset(S2[:, W - 1:], 0.0)
                # Podd = max(EH0, ED1*evenrow)
                nc.vector.scalar_tensor_tensor(S[:], S2[:], emask[:, 0:1],
                                               S[:], op0=AL.mult, op1=AL.max)
                # Pu = max(Podd, shiftup(Podd)) (even rows valid)
                nc.sync.dma_start(S2[0:127, :], S[1:128, :])
                nc.vector.tensor_tensor(S[0:127, :], S[0:127, :], S2[0:127, :],
                                        op=AL.max)
                # X[:, J<w1-1] = max(Pu[:, 2J+1], ED2[:, 2J+2]);
                # ED2[:, 2J+2] = g[:, 2J+2] * gup[:, 2J+1]
                nc.vector.tensor_tensor(S2[:, :w1 - 1], g[:, 2:W:2],
                                        gup[:, 1:W - 1:2], op=AL.mult)
                nc.vector.tensor_tensor(S2[:, :w1 - 1], S2[:, :w1 - 1],
                                        S[:, 1:W - 2:2], op=AL.max)
                nc.vector.memset(S2[:, w1 - 1:w1], 0.0)
                # compact even rows -> EH1 stage rows [64b:64b+64)
                nc.sync.dma_start(
                    self.L1R[b // 2][64 * (b % 2):64 * (b % 2) + 64, :],
                    S2[0:128:2, :w1])
            # gh1f = BIG*(1 - EH1 shifted right by one coarse col)
            for i in range(2):
                nc.vector.tensor_scalar(self.gh1f[i][:, 1:],
                                        self.L1R[i][:, :w1 - 1], -BIG, BIG,
                                        op0=AL.mult, op1=AL.add)
                nc.vector.memset(self.gh1f[i][:, 0:1], BIG)
            # ---- EV1 terms (re-unpack, rolling) ----
            g_next = unpack(0)
            for b in range(YT):
                g = g_next
                g_next = unpack(b + 1) if b + 1 < YT else None
                gup = pp.tile([128, W], U8, tag="gup", name="gup")
                nc.sync.dma_start(gup[0:127, :], g[1:128, :])
                if g_next is not None:
                    nc.sync.dma_start(gup[127:128, :], g_next[0:1, :])
                else:
                    nc.sync.dma_start(gup[127:128, :], zrow8[:])
                S = self.S()
                S2 = self.S2()
                # EV0 = g & gup
                nc.vector.tensor_tensor(S[:], g[:], gup[:], op=AL.mult)
                # t1 = max(EV0[ev], ED1[ev]); ED1[2J] = g[2J]*gup[2J+1]
                nc.vector.tensor_tensor(S2[:, :w1], g[:, 0:W:2],
                                        gup[:, 1:W:2], op=AL.mult)
                nc.vector.tensor_tensor(S2[:, :w1], S2[:, :w1], S[:, 0:W:2],
                                        op=AL.max)
                # t2 = max(EV0[odd], ED2[odd]); ED2[2J+1] = g[2J+1]*gup[2J]
                nc.vector.tensor_tensor(S2[:, w1:2 * w1], g[:, 1:W:2],
                                        gup[:, 0:W:2], op=AL.mult)
                nc.vector.tensor_tensor(S2[:, w1:2 * w1], S2[:, w1:2 * w1],
                                        S[:, 1:W:2], op=AL.max)
                # Vfine = max(t1, t2) (odd rows valid)
                nc.vector.tensor_tensor(S2[:, :w1], S2[:, :w1],
                                        S2[:, w1:2 * w1], op=AL.max)
                # compact odd rows, shifted down one coarse row:
                # gv row (64b + q + 1) <- Vfine fine row 2q+1
                lo = 64 * b + 1
                hi = min(64 * b + 65, r1)
                n1 = min(hi, 128) - lo if lo < 128 else 0
                if n1 > 0:
                    nc.sync.dma_start(self.L1R[0][lo:lo + n1, :],
                                      S2[1:2 * n1:2, :w1])
                if hi > 128:
                    lo2 = max(lo, 128)
                    q0 = lo2 - (64 * b + 1)
                    n2 = hi - lo2
                    nc.sync.dma_start(
                        self.L1R[1][lo2 - 128:lo2 - 128 + n2, :],
                        S2[2 * q0 + 1:2 * (q0 + n2):2, :w1])
            # gv1f = BIG*(1 - stage); row0 = BIG
            for i in range(2):
                nc.vector.tensor_scalar(self.gv1f[i][:, :], self.L1R[i][:, :],
                                        -BIG, BIG, op0=AL.mult, op1=AL.add)
            nc.vector.memset(self.gv1f[0][0:1, :], BIG)

    # ------------------------------------------------------------------
    def l0_rep(self):
        """Jacobi 3x3 masked min sweep + fwd/bwd masked row scans."""
        nc = self.nc
        R0, GBb = self.R0, self.GBb
        rs = self.rowsnap
        for b in range(YT):
            nc.sync.dma_start(rs[2 * b:2 * b + 1, :], R0[b][0:1, :])
            nc.sync.dma_start(rs[2 * b + 1:2 * b + 2, :], R0[b][127:128, :])
        for b in range(YT):
            S = self.S()
            nc.sync.dma_start(S[0:127, :], R0[b][1:128, :])
            if b + 1 < YT:
                nc.sync.dma_start(S[127:128, :],
                                  rs[2 * (b + 1):2 * (b + 1) + 1, :])
            else:
                nc.sync.dma_start(S[127:128, :], self.bigrow[:])
            nc.vector.tensor_tensor(S[:], R0[b][:], S[:], op=AL.min)
            S2 = self.S2()
            nc.sync.dma_start(S2[1:128, :], R0[b][0:127, :])
            if b > 0:
                nc.sync.dma_start(S2[0:1, :],
                                  rs[2 * (b - 1) + 1:2 * (b - 1) + 2, :])
            else:
                nc.sync.dma_start(S2[0:1, :], self.bigrow[:])
            nc.vector.tensor_tensor(S[:], S[:], S2[:], op=AL.min)
            nc.vector.tensor_tensor(R0[b][:, 1:], S[:, 1:], S[:, :-1],
                                    op=AL.min)
            nc.vector.tensor_copy(R0[b][:, 0:1], S[:, 0:1])
            nc.vector.tensor_tensor(R0[b][:, :-1], R0[b][:, :-1], S[:, 1:],
                                    op=AL.min)
            nc.vector.tensor_tensor(R0[b][:], R0[b][:], GBb[b][:], op=AL.max)
            nc.vector.tensor_tensor_scan(R0[b][:], GBb[b][:], R0[b][:], BIG,
                                         op0=AL.max, op1=AL.min)
            nc.vector.tensor_tensor_scan(R0[b][:, ::-1], GBb[b][:, ::-1],
                                         R0[b][:, ::-1], BIG,
                                         op0=AL.max, op1=AL.min)

    def lev_smooth(self, k):
        """One rep: seg row scans fwd+bwd, then gated +-1 vertical."""
        nc = self.nc
        rk, wk = LEV[k]
        Rt = self.lab[k]
        pR = min(rk, 128)
        n = len(Rt)
        gh = self.gh1f if k == 1 else self.Hf[k]
        gv = self.gv1f if k == 1 else self.Vf[k]
        for i in range(n):
            nc.vector.tensor_tensor_scan(Rt[i][:, :], gh[i][:, :], Rt[i][:, :],
                                         BIG, op0=AL.max, op1=AL.min)
            nc.vector.tensor_tensor_scan(
                Rt[i][:, wk - 2::-1], gh[i][:, wk - 1:0:-1],
                Rt[i][:, wk - 2::-1], Rt[i][:, wk - 1:wk],
                op0=AL.max, op1=AL.min)
        S = self.S()
        for i in range(n):
            sl = S[:pR, i * wk:(i + 1) * wk]
            nc.sync.dma_start(sl[1:pR, :], Rt[i][0:pR - 1, :])
            if i > 0:
                nc.sync.dma_start(sl[0:1, :], Rt[i - 1][pR - 1:pR, :])
            else:
                nc.sync.dma_start(sl[0:1, :], self.bigrow[:, :wk])
        for i in range(n):
            sl = S[:pR, i * wk:(i + 1) * wk]
            nc.vector.tensor_tensor(sl, sl, gv[i][:, :], op=AL.add)
            nc.vector.tensor_tensor(Rt[i][:, :], Rt[i][:, :], sl, op=AL.min)
        S2 = self.S2()
        for i in range(n):
            u = S2[:pR, i * wk:(i + 1) * wk]
            nc.vector.tensor_tensor(u, Rt[i][:, :], gv[i][:, :], op=AL.add)
        S = self.S()
        for i in range(n):
            sl = S[:pR, i * wk:(i + 1) * wk]
            nc.sync.dma_start(sl[0:pR - 1, :], S2[1:pR, i * wk:(i + 1) * wk])
            if i + 1 < n:
                nc.sync.dma_start(sl[pR - 1:pR, :],
                                  S2[0:1, (i + 1) * wk:(i + 2) * wk])
            else:
                nc.sync.dma_start(sl[pR - 1:pR, :], self.bigrow[:, :wk])
            nc.vector.tensor_tensor(Rt[i][:, :], Rt[i][:, :], sl, op=AL.min)

    def restrict01(self):
        nc = self.nc
        r1, w1 = LEV[1]
        for b in range(YT):
            S2 = self.S2()
            nc.vector.tensor_tensor(S2[:, :w1], self.R0[b][:, 0:W:2],
                                    self.R0[b][:, 1:W:2], op=AL.min)
            S = self.S()
            nc.sync.dma_start(S[0:127, :w1], S2[1:128, :w1])
            nc.vector.tensor_tensor(S2[:, :w1], S2[:, :w1], S[:, :w1],
                                    op=AL.min)
            nc.sync.dma_start(
                self.L1R[b // 2][64 * (b % 2):64 * (b % 2) + 64, :],
                S2[0:128:2, :w1])

    def deep_down(self, k):
        nc = self.nc
        rf, wf = LEV[k - 1]
        rk, wk = LEV[k]
        pRf = min(rf, 128)
        nRf = (rf + 127) // 128
        pRfh = pRf // 2
        Rf = self.lab[k - 1]
        Mk = self.lab[k][0]
        ghf = self.gh1f if k == 2 else self.Hf[k - 1]
        gvf = self.gv1f if k == 2 else self.Vf[k - 1]
        # restrict
        for i in range(nRf):
            S2 = self.S2()
            nc.vector.tensor_tensor(S2[:pRf, :wk], Rf[i][:, 0:wf:2],
                                    Rf[i][:, 1:wf:2], op=AL.min)
            S = self.S()
            nc.sync.dma_start(S[0:pRf - 1, :wk], S2[1:pRf, :wk])
            nc.vector.tensor_tensor(S2[:pRf, :wk], S2[:pRf, :wk],
                                    S[:pRf, :wk], op=AL.min)
            nc.sync.dma_start(Mk[i * pRfh:(i + 1) * pRfh, :],
                              S2[0:pRf:2, :wk])
        nc.vector.tensor_copy(self.snapM[k][:, :], Mk[:, :])
        # gates
        S2g = self.S2()
        for i in range(nRf):
            S = self.S()
            up = S[:pRf, 2 * wk:3 * wk]
            nc.sync.dma_start(up[0:pRf:2, :],
                              self.snapM[k][i * pRfh:(i + 1) * pRfh, :])
            nc.sync.dma_start(up[1:pRf:2, :],
                              self.snapM[k][i * pRfh:(i + 1) * pRfh, :])
            nef = (S2g[:pRf, i * wf:(i + 1) * wf] if nRf > 1
                   else S2g[:pRf, :wf])
            nc.vector.tensor_tensor(nef, Rf[i][:, :], dbl(up),
                                    op=AL.not_equal)
            # Hf: Aev[:, J] = gf[:, 2J] + BIG*(nef[:, 2J-1] + nef[:, 2J])
            Aev = S[:pRf, 0:wk]
            Sv = S[:pRf, wk:2 * wk]
            nc.vector.tensor_tensor(Aev[:, 1:wk], nef[:, 1:wf - 2:2],
                                    nef[:, 2:wf:2], op=AL.add)
            nc.vector.scalar_tensor_tensor(Aev[:, 1:wk], Aev[:, 1:wk], BIG,
                                           ghf[i][:, 2:wf:2],
                                           op0=AL.mult, op1=AL.add)
            nc.vector.memset(Aev[:, 0:1], BIG)
            nc.sync.dma_start(Sv[0:pRf - 1, :], Aev[1:pRf, :])
            nc.vector.tensor_tensor(Aev, Aev, Sv, op=AL.min)
            nc.sync.dma_start(self.Hf[k][0][i * pRfh:(i + 1) * pRfh, :],
                              Aev[0:pRf:2, :])
        # Vf: B = gvf + BIG*(nef_up + nef); x-pair min; row compact
        for i in range(nRf):
            nef = (S2g[:pRf, i * wf:(i + 1) * wf] if nRf > 1
                   else S2g[:pRf, :wf])
            S = self.S()
            Sv = S[:pRf, 0:wf]
            nc.sync.dma_start(Sv[1:pRf, :], nef[0:pRf - 1, :])
            if i > 0:
                pnef = S2g[:pRf, (i - 1) * wf:i * wf]
                nc.sync.dma_start(Sv[0:1, :], pnef[pRf - 1:pRf, :])
            nc.vector.tensor_tensor(Sv, Sv, nef, op=AL.add)
            nc.vector.scalar_tensor_tensor(Sv, Sv, BIG, gvf[i][:, :],
                                           op0=AL.mult, op1=AL.add)
            Bp = S[:pRf, wf:wf + wk]
            nc.vector.tensor_tensor(Bp, Sv[:, 0:wf:2], Sv[:, 1:wf:2],
                                    op=AL.min)
            nc.sync.dma_start(self.Vf[k][0][i * pRfh:(i + 1) * pRfh, :],
                              Bp[0:pRf:2, :])
        nc.vector.memset(self.Vf[k][0][0:1, :], BIG)
        for _ in range(NS1):
            self.lev_smooth(k)

    def deep_up(self, k):
        nc = self.nc
        rf, wf = LEV[k - 1]
        rk, wk = LEV[k]
        pRf = min(rf, 128)
        nRf = (rf + 127) // 128
        pRfh = pRf // 2
        Rf = self.lab[k - 1]
        Mk = self.lab[k][0]
        for i in range(nRf):
            S = self.S()
            upl = S[:pRf, 0:wk]
            ups = S[:pRf, wk:2 * wk]
            nc.sync.dma_start(upl[0:pRf:2, :],
                              Mk[i * pRfh:(i + 1) * pRfh, :])
            nc.sync.dma_start(upl[1:pRf:2, :],
                              Mk[i * pRfh:(i + 1) * pRfh, :])
            nc.sync.dma_start(ups[0:pRf:2, :],
                              self.snapM[k][i * pRfh:(i + 1) * pRfh, :])
            nc.sync.dma_start(ups[1:pRf:2, :],
                              self.snapM[k][i * pRfh:(i + 1) * pRfh, :])
            ne = self.S2()
            nc.vector.tensor_tensor(ne[:pRf, :wf], Rf[i][:, :], dbl(ups),
                                    op=AL.not_equal)
            nc.vector.scalar_tensor_tensor(ne[:pRf, :wf], ne[:pRf, :wf], BIG,
                                           dbl(upl), op0=AL.mult, op1=AL.add)
            nc.vector.tensor_tensor(Rf[i][:, :], Rf[i][:, :], ne[:pRf, :wf],
                                    op=AL.min)
        if k - 1 >= 2:
            for _ in range(NS1):
                self.lev_smooth(k - 1)

    def prolong10(self):
        nc = self.nc
        r1, w1 = LEV[1]
        for b in range(YT):
            up = self.S()
            src = self.L1R[b // 2][64 * (b % 2):64 * (b % 2) + 64, :]
            nc.sync.dma_start(up[0:128:2, :w1], src)
            nc.sync.dma_start(up[1:128:2, :w1], src)
            nc.vector.tensor_tensor(self.R0[b][:], self.R0[b][:],
                                    dbl(up[:, :w1]), op=AL.min)
            nc.vector.tensor_tensor(self.R0[b][:], self.R0[b][:],
                                    self.GBb[b][:], op=AL.max)

    def cycle(self, c, chga):
        nc = self.nc
        for _ in range(NS0):
            self.l0_rep()
        self.restrict01()
        for _ in range(NS1):
            self.lev_smooth(1)
        for k in range(2, NLEV):
            self.deep_down(k)
        for k in range(NLEV - 1, 1, -1):
            self.deep_up(k)
        self.prolong10()
        self.l0_rep()
        if c < self.ncyc - 1:
            # change tracking only needed for the certificate (final cycle);
            # refresh prevt just before it so the comparison is right
            if c == self.ncyc - 2:
                for b in range(YT):
                    nc.sync.dma_start(self.prevt[b][:], self.R0[b][:])
            return
        for b in range(YT):
            pv = self.S()
            nc.sync.dma_start(pv[:], self.prevt[b][:])
            ne = self.S2()
            nc.vector.tensor_tensor(ne[:], self.R0[b][:], pv[:],
                                    op=AL.not_equal)
            red = self.scr.tile([128, 1], F32, tag="red", name="red")
            nc.vector.tensor_reduce(red[:], ne[:], axis=AX.X, op=AL.add)
            nc.vector.tensor_tensor(chga[:, c:c + 1], chga[:, c:c + 1],
                                    red[:], op=AL.add)
            nc.sync.dma_start(self.prevt[b][:], self.R0[b][:])


def build_program(ncyc=NCYC):
    nc = bacc.Bacc("TRN2", target_bir_lowering=False, debug=False,
                   num_devices=NCORES)
    ins = {
        "blob": nc.dram_tensor("blob", [SR, WP], U8,
                               kind="ExternalInput").ap(),
    }
    outs = {
        "out": nc.dram_tensor("out", [1, 3 * SR * W + 512 * ncyc], U8,
                              kind="ExternalOutput").ap(),
    }
    with tile.TileContext(nc) as tc:
        Dev(tc, ins, outs, ncyc).build()
    nc.compile()
    return nc


# ---------------------------------------------------------------------------
# host side
# ---------------------------------------------------------------------------

def make_blob(f):
    return np.packbits(f, axis=1)


def decode_out(o):
    """o: u8 flat device output -> (labels int32 [SR, W], chg [128, NCYC])."""
    SW = SR * W
    p01 = o[:2 * SW].view(np.uint16).reshape(SR, W)
    p2 = o[2 * SW:3 * SW].reshape(SR, W)
    lab = p01.astype(np.int32) | (p2.astype(np.int32) << 16)
    chg = o[3 * SW:3 * SW + 512 * NCYC].view(np.int32).reshape(128, NCYC)
    return lab, chg


def seam_classes(labs):
    """labs: per-strip LOCAL label arrays. Union-find over seam equivalences.
    Returns (cu, cv): sorted GLOBAL labels that change -> new global value."""
    pairs_a, pairs_b = [], []
    for c in range(NCORES - 1):
        A = labs[c][SR - 1].astype(np.int64)
        B = labs[c + 1][0].astype(np.int64)
        A = np.where(A > 0, A + c * SR * W, 0)
        B = np.where(B > 0, B + (c + 1) * SR * W, 0)
        for sh in (-1, 0, 1):
            Bs = np.roll(B, sh)
            valid = (A > 0) & (Bs > 0)
            if sh == 1:
                valid[0] = False
            if sh == -1:
                valid[-1] = False
            pairs_a.append(A[valid])
            pairs_b.append(Bs[valid])
    ea = np.concatenate(pairs_a)
    eb = np.concatenate(pairs_b)
    if len(ea) == 0:
        return np.empty(0, np.int64), np.empty(0, np.int32)
    u = np.unique(np.concatenate([ea, eb]))
    ia = np.searchsorted(u, ea)
    ib = np.searchsorted(u, eb)
    val = u.copy()
    for _ in range(100):
        old = val.copy()
        nv = val.copy()
        np.minimum.at(nv, ia, val[ib])
        np.minimum.at(nv, ib, val[ia])
        nv = np.minimum(nv, nv[np.searchsorted(u, nv)])
        val = nv
        if np.array_equal(val, old):
            break
    ch = val != u
    return u[ch], val[ch].astype(np.int32)


# -- numpy multigrid fallback (never triggers when the device converges) --

_K64 = np.int64(2 ** 26)


def _seg_scan(X, G, axis, reverse=False):
    if reverse:
        X = np.flip(X, axis=axis); G = np.flip(G, axis=axis)
    brk = G >= BIGI
    seg = np.cumsum(brk, axis=axis).astype(np.int64)
    sp = (X.shape[axis] + 2) - seg
    C = np.minimum.accumulate(X + sp * _K64, axis=axis)
    res = np.minimum(C - sp * _K64, X)
    if reverse:
        res = np.flip(res, axis=axis)
    return res


def _sweep3(lab, fg):
    h, w = lab.shape
    p = np.full((h + 2, w + 2), BIGI)
    p[1:-1, 1:-1] = lab
    m = lab.copy()
    for di in range(3):
        for dj in range(3):
            m = np.minimum(m, p[di:di + h, dj:dj + w])
    return np.where(fg, m, BIGI)


def _lscan(T, gh, gv):
    Hb = np.full(gh.shape, BIGI); Hb[:, :-1] = gh[:, 1:]
    Vb = np.full(gv.shape, BIGI); Vb[:-1, :] = gv[1:, :]
    T = _seg_scan(T, gh, 1)
    T = _seg_scan(T, Hb, 1, reverse=True)
    T = _seg_scan(T, gv, 0)
    return _seg_scan(T, Vb, 0, reverse=True)


def _restr(lab):
    return np.minimum(np.minimum(lab[0::2, 0::2], lab[0::2, 1::2]),
                      np.minimum(lab[1::2, 0::2], lab[1::2, 1::2]))


def _np_gates_l1(f):
    def q(A, i, j):
        return A[i::2, j::2]
    EH0 = f & np.roll(f, -1, 1); EH0[:, -1] = False
    EV0 = f & np.roll(f, -1, 0); EV0[-1, :] = False
    ED1 = f & np.roll(np.roll(f, -1, 0), -1, 1)
    ED1[-1, :] = False; ED1[:, -1] = False
    ED2 = f & np.roll(np.roll(f, -1, 0), 1, 1)
    ED2[-1, :] = False; ED2[:, 0] = False
    EH1 = (q(EH0, 0, 1) | q(EH0, 1, 1) | q(ED1, 0, 1)
           | q(np.roll(ED2, -2, 1), 0, 0))
    EH1[:, -1] = False
    EV1 = q(EV0, 1, 0) | q(EV0, 1, 1) | q(ED1, 1, 0) | q(ED2, 1, 1)
    EV1[-1, :] = False
    s2, w2 = f.shape[0] // 2, f.shape[1] // 2
    gh = np.full((s2, w2), BIGI); gh[:, 1:] = np.where(EH1[:, :-1], 0, BIGI)
    gv = np.full((s2, w2), BIGI); gv[1:, :] = np.where(EV1[:-1, :], 0, BIGI)
    return gh, gv


def _np_coarse_gates(gh, gv, nef):
    shp = (gh.shape[0] // 2, gh.shape[1] // 2)
    Hf = np.full(shp, BIGI)
    t1 = gh[0::2, 0::2] + (np.roll(nef[0::2, 1::2], 1, 1) + nef[0::2, 0::2]) * BIGI
    t2 = gh[1::2, 0::2] + (np.roll(nef[1::2, 1::2], 1, 1) + nef[1::2, 0::2]) * BIGI
    Hf[:, 1:] = np.minimum(t1, t2)[:, 1:]
    Vf = np.full(shp, BIGI)
    t1 = gv[0::2, 0::2] + (np.roll(nef[1::2, 0::2], 1, 0) + nef[0::2, 0::2]) * BIGI
    t2 = gv[0::2, 1::2] + (np.roll(nef[1::2, 1::2], 1, 0) + nef[0::2, 1::2]) * BIGI
    Vf[1:, :] = np.minimum(t1, t2)[1:, :]
    return Hf, Vf


def _host_finish_local(lab, fg):
    """Warm-start numpy multigrid (full column scans) to the strip fixpoint.
    Device labels are sound (monotone upper bounds whose values are member
    indices), so continuing from them is valid; the loop ends on a no-change
    cycle, whose leading 3x3 sweep certifies exactness."""
    lab = np.where(fg & (lab > 0), lab.astype(np.int64), BIGI)
    gadd = np.where(fg, 0, BIGI).astype(np.int64)
    gh1, gv1 = _np_gates_l1(fg)
    for _ in range(200):
        new = _sweep3(lab, fg)
        new = _seg_scan(new, gadd, 1)
        new = _seg_scan(new, gadd, 1, reverse=True)
        labs = {1: _restr(new)}
        gh = {1: gh1}; gv = {1: gv1}
        snaps = {}
        for _r in range(2):
            labs[1] = _lscan(labs[1], gh1, gv1)
        for k in range(2, 6):
            fine = labs[k - 1]
            Lmin = _restr(fine)
            snaps[k] = Lmin
            nef = (fine != np.repeat(np.repeat(Lmin, 2, 0), 2, 1)).astype(np.int64)
            gh[k], gv[k] = _np_coarse_gates(gh[k - 1], gv[k - 1], nef)
            cur = Lmin.copy()
            for _r in range(2):
                cur = _lscan(cur, gh[k], gv[k])
            labs[k] = cur
        for k in range(5, 1, -1):
            up = np.repeat(np.repeat(labs[k], 2, 0), 2, 1)
            upm = np.repeat(np.repeat(snaps[k], 2, 0), 2, 1)
            labs[k - 1] = np.minimum(labs[k - 1],
                                     up + (labs[k - 1] != upm) * BIGI)
            if k - 1 >= 2:
                for _r in range(2):
                    labs[k - 1] = _lscan(labs[k - 1], gh[k - 1], gv[k - 1])
        up = np.repeat(np.repeat(labs[1], 2, 0), 2, 1)
        new = np.where(fg, np.minimum(new, up), BIGI)
        new = _sweep3(new, fg)
        new = _seg_scan(new, gadd, 1)
        new = _seg_scan(new, gadd, 1, reverse=True)
        if np.array_equal(new, lab):
            break
        lab = new
    return np.where(lab >= BIGI, 0, lab).astype(np.int32)


_CACHED = {}


def kernel(prob):
    prob2 = np.squeeze(np.asarray(prob))
    fg = prob2 > 0.5

    if 'nc' not in _CACHED:
        _CACHED['nc'] = build_program()
    nc = _CACHED['nc']

    in_maps = [{"blob": make_blob(fg[c * SR:(c + 1) * SR])}
               for c in range(NCORES)]
    res = run_bass_kernel_spmd(nc, in_maps, core_ids=list(range(NCORES)))
    kernel._launches = 1
    labs = []
    chgs = []
    for c in range(NCORES):
        lab, chg = decode_out(res.results[c]["out"][0])
        labs.append(lab)
        chgs.append(chg.sum(axis=0))
    kernel._chgs = np.stack(chgs)
    for c in range(NCORES):
        if kernel._chgs[c, -1] != 0:
            # fallback: finish this strip on host (local labels stay local)
            f = fg[c * SR:(c + 1) * SR]
            labs[c] = _host_finish_local(labs[c], f)
    cu, cv = seam_classes(labs)
    # per-strip local LUT (strip-local labels <= SR*W) -> one take per strip
    full = np.empty((H, W), np.int32)
    for c in range(NCORES):
        off = np.int64(c * SR * W)
        lut = np.arange(off, off + SR * W + 1, dtype=np.int32)
        lut[0] = 0
        if len(cu):
            m = (cu > off) & (cu <= off + SR * W)
            lut[(cu[m] - off).astype(np.int64)] = cv[m]
        full[c * SR:(c + 1) * SR] = lut.take(labs[c])
    return full


if __name__ == '__main__':
    import time
    mode = sys.argv[1] if len(sys.argv) > 1 else 'e2e'
    fgA = np.load('/root/problem/work/fg.npy')
    if mode == 'parity':
        ncyc = int(sys.argv[2]) if len(sys.argv) > 2 else 1
        ncb = build_program(ncyc)
        in_maps = [{"blob": make_blob(fgA[c * SR:(c + 1) * SR])}
                   for c in range(NCORES)]
        t0 = time.time()
        res = run_bass_kernel_spmd(ncb, in_maps, core_ids=list(range(NCORES)))
        print(f"launch: {time.time() - t0:.1f}s", flush=True)
        sys.path.insert(0, '/root/problem')
        from model import cycle as mcycle, make_l1_gates
        from model import BIGI as MBIGI
        for c in range(NCORES):
            f = fgA[c * SR:(c + 1) * SR]
            idx = np.arange(SR * W, dtype=np.int64).reshape(SR, W) + 1
            mlab = np.where(f, idx, MBIGI)
            gadd = np.where(f, 0, MBIGI).astype(np.int64)
            gh1, gv1 = make_l1_gates(f)
            mchg = []
            for _ in range(ncyc):
                new = mcycle(mlab, f, gadd, gh1, gv1, l0col=False, nlev=6,
                             vml=True, ns0=NS0, ns1=NS1)
                mchg.append(int((new != mlab).sum()))
                mlab = new
            mres = np.where(mlab >= MBIGI, 0, mlab).astype(np.int32)
            oo = res.results[c]["out"][0]
            SW = SR * W
            p01 = oo[:2 * SW].view(np.uint16).reshape(SR, W)
            p2 = oo[2 * SW:3 * SW].reshape(SR, W)
            got = p01.astype(np.int32) | (p2.astype(np.int32) << 16)
            dchg = (oo[3 * SW:3 * SW + 512 * ncyc].view(np.int32)
                    .reshape(128, ncyc).sum(axis=0).astype(np.int64))
            eq = np.array_equal(got, mres)
            print(f"core {c}: equal={eq} dev_chg={list(dchg)} model_chg={mchg}")
            if not eq:
                bad = np.argwhere(got != mres)
                print(f"  nbad={len(bad)} first={bad[:5].tolist()}")
                for y, x in bad[:5]:
                    print(f"   ({y},{x}) got={got[y, x]} exp={mres[y, x]} "
                          f"fg={f[y, x]}")
                break
    else:
        prob = np.where(fgA, 0.9, 0.1).astype(np.float32)[None, None]
        exp = np.load('/root/problem/work/ref_out.npy')
        t0 = time.time()
        out = kernel(prob)
        print(f"kernel cold: {time.time() - t0:.2f}s", flush=True)
        print("equal:", np.array_equal(out, exp))
        print("chg last2:", kernel._chgs[:, -2:].astype(int).tolist())
        for _ in range(2):
            t0 = time.time()
            out = kernel(prob)
            print(f"kernel warm: {time.time() - t0:.2f}s "
                  f"equal={np.array_equal(out, exp)}")


# revision 11
# speedup vs baseline: 1.0918x; 1.0918x over previous
"""Trainium2 Bass kernel: 8-connectivity CCL of a 4096x4096 binary image
(prob > 0.5); labels = min linear index in component + 1, background 0.

Single-launch, transpose-free design: image split into 8 row-strips of 512
rows, one per NeuronCore. Each core solves its strip to a LOCAL fixpoint on
device with a 6-level multigrid min-propagation scheme, all arrays kept in
row-major form; vertical data movement (3x3 sweep, 2x2 restriction,
prolongation, gated +-1 vertical steps, nef-gate assembly) is done with
partition-shifted / partition-strided SBUF-to-SBUF DMAs, so only the DMA
and DVE(vector) engines are used. The L1 block-edge gates are built on
device from the foreground mask. NCYC unrolled V-cycles; each cycle starts
with an exact Jacobi 3x3 masked min sweep, so "last cycle changed nothing"
(checked via per-cycle change counts) certifies strip-exact labels. Host
then merges the 7 seam equivalences (tiny union-find) and remaps.

Input per core: bit-packed foreground u8 [512, 512] (np.packbits of
prob > 0.5, bit 1 = foreground). Output per core: flat int32
[1, 512*4096 + 128*NCYC]: labels row-major (0 = bg, strip-local values),
then per-cycle change counts [128, NCYC].
"""
import sys
sys.path.insert(0, '/opt/trn_rl_repo')
sys.path.insert(0, '/root/.axon_site')
sys.path.insert(0, '/root/.axon_site/_ro/trn_rl_repo')
import numpy as np
from contextlib import ExitStack

import concourse.bass as bass
import concourse.bacc as bacc
import concourse.mybir as mybir
import concourse.tile as tile
from concourse.bass_utils import run_bass_kernel_spmd

F32 = mybir.dt.float32
I32 = mybir.dt.int32
U8 = mybir.dt.uint8
BF16 = mybir.dt.bfloat16
U16 = mybir.dt.uint16
AL = mybir.AluOpType
AX = mybir.AxisListType

H = W = 4096
NCORES = 8
SR = H // NCORES            # 512
YT = SR // 128              # 4
WP = W // 8                 # packed bytes per row
BIG = float(2 ** 25)
BIGI = np.int64(2 ** 25)
NCYC = 13
NS0 = 2
NS1 = 2
NLEV = 6
LEV = {k: (SR >> k, W >> k) for k in range(NLEV)}


def dbl(ap):
    """stride-0 double the last free dim: [p, n] -> reads each elem twice"""
    return ap.unsqueeze(2).broadcast_to([ap.shape[0], ap.shape[1], 2])


class Dev:
    def __init__(self, tc, ins, outs, ncyc):
        self.tc = tc
        self.nc = tc.nc
        self.ins = ins
        self.outs = outs
        self.ncyc = ncyc

    def S(self):
        return self.scr.tile([128, W], F32, tag="S", name="S")

    def S2(self):
        return self.scr.tile([128, W], F32, tag="S2", name="S2")

    def build(self):
        nc, tc = self.nc, self.tc
        ctx = ExitStack()
        with ctx:
            pers = ctx.enter_context(tc.tile_pool(name="pers", bufs=1))
            self.scr = ctx.enter_context(tc.tile_pool(name="scr", bufs=1))
            dpool = ctx.enter_context(
                tc.tile_pool(name="dscratch", bufs=1, space="DRAM"))

            R0 = [pers.tile([128, W], F32, tag=f"R0_{b}", name=f"R0_{b}")
                  for b in range(YT)]
            GBb = [pers.tile([128, W], BF16, tag=f"GB_{b}", name=f"GB_{b}")
                   for b in range(YT)]
            r1, w1 = LEV[1]
            gh1s = pers.tile([128, 2 * w1], BF16, tag="gh1f", name="gh1f")
            gv1s = pers.tile([128, 2 * w1], BF16, tag="gv1f", name="gv1f")
            gh1f = [gh1s[:, i * w1:(i + 1) * w1] for i in range(2)]
            gv1f = [gv1s[:, i * w1:(i + 1) * w1] for i in range(2)]
            L1R = [pers.tile([128, w1], F32, tag=f"L1R_{i}", name=f"L1R_{i}")
                   for i in range(2)]
            chga = pers.tile([128, self.ncyc], F32, tag="chga", name="chga")
            nc.vector.memset(chga[:], 0.0)
            self.R0, self.GBb, self.L1R = R0, GBb, L1R
            self.gh1f, self.gv1f = gh1f, gv1f

            self.lab = {1: L1R}
            self.Hf = {}
            self.Vf = {}
            self.snapM = {}
            for k in range(2, NLEV):
                rk, wk = LEV[k]
                self.lab[k] = [pers.tile([rk, wk], F32, tag=f"lab{k}",
                                         name=f"lab{k}")]
                self.Hf[k] = [pers.tile([rk, wk], F32, tag=f"Hf{k}",
                                        name=f"Hf{k}")]
                self.Vf[k] = [pers.tile([rk, wk], F32, tag=f"Vf{k}",
                                        name=f"Vf{k}")]
                self.snapM[k] = pers.tile([rk, wk], F32, tag=f"snapM{k}",
                                          name=f"snapM{k}")

            self.prevt = [dpool.tile([128, W], F32, tag=f"prevd_{b}",
                                     name=f"prevd_{b}") for b in range(YT)]
            self.bigrow = dpool.tile([1, W], F32, tag="bigrowd",
                                     name="bigrowd")
            self.rowsnap = dpool.tile([8, W], F32, tag="rowsnap",
                                      name="rowsnap")
            zrow8 = dpool.tile([1, W], U8, tag="zrow8", name="zrow8")

            blob_r = self.ins["blob"].rearrange("(a p) w -> a p w", p=128)

            # ---- prologue: constants ----
            bs = self.S()
            nc.vector.memset(bs[0:1, :], BIG)
            nc.sync.dma_start(self.bigrow[:], bs[0:1, :])
            mk = self.scr.tile([128, 1], F32, tag="red", name="mk")
            mkv = mk.bitcast(I32)[:, 0:1]
            nc.gpsimd.iota(mkv, pattern=[[1, 1]], base=0, channel_multiplier=1)
            nc.vector.tensor_scalar(mkv, mkv, 1.0, None, op0=AL.bitwise_and)
            emask = pers.tile([128, 1], F32, tag="emask", name="emask")
            nc.vector.tensor_scalar(emask[:], mkv, -1.0, 1.0, op0=AL.mult,
                                    op1=AL.add)

            # ---- prologue: unpack fg + build gates (device-side) ----
            self._build_gates(blob_r, zrow8, emask)

            # ---- prologue: init labels ----
            for b in range(YT):
                ti = self.S()
                tiv = ti.bitcast(I32)[:, :W]
                nc.gpsimd.iota(tiv, pattern=[[1, W]], base=b * 128 * W + 1,
                               channel_multiplier=W)
                nc.vector.tensor_copy(R0[b][:], tiv)
                t1 = self.S2()
                nc.vector.tensor_scalar(t1[:], R0[b][:], -1.0, BIG,
                                        op0=AL.mult, op1=AL.add)
                nc.vector.scalar_tensor_tensor(t1[:], GBb[b][:], 1.0 / BIG,
                                               t1[:], op0=AL.mult,
                                               op1=AL.mult)
                nc.vector.tensor_tensor(R0[b][:], R0[b][:], t1[:], op=AL.add)
                nc.sync.dma_start(self.prevt[b][:], R0[b][:])

            # ---- cycles ----
            for c in range(self.ncyc):
                self.cycle(c, chga)

            # ---- epilogue: e = lab-1 (bg don't-care, host masks by fg);
            # u16 low plane + 8x5-bit highs packed into 5 bytes. ----
            SW = SR * W
            G8 = W // 8
            for b in range(YT):
                ne = self.S()
                nev = ne.bitcast(F32)[:, :W]
                nc.vector.tensor_scalar(nev, R0[b][:], BIG, 0.0,
                                        op0=AL.is_lt, op1=AL.add)
                oi = self.S2()
                oiv = oi.bitcast(I32)[:, :W]
                nc.vector.tensor_tensor(oiv, R0[b][:], nev, op=AL.mult)
                nc.vector.tensor_scalar(oiv, oiv, 1.0, None, op0=AL.subtract)
                d16 = self.outs["out"][0:1, b * 128 * 2 * W:
                                       (b + 1) * 128 * 2 * W]
                d16r = d16.rearrange("o (p w) -> (o p) w",
                                     w=2 * W).bitcast(U16)
                for hh in range(2):
                    nc.sync.dma_start(
                        d16r[:, hh * (W // 2):(hh + 1) * (W // 2)],
                        oi.bitcast(U16)[:, hh * W:hh * W + W:2])
                # highs: e >> 16 (5 bits on fg), extract to u8, pack 8->5
                nc.vector.tensor_scalar(oiv, oiv, 16.0, None,
                                        op0=AL.logical_shift_right)
                hu = ne.bitcast(U8)[:, 0:W]
                nc.sync.dma_start(hu, oi.bitcast(U8)[:, 0:4 * W:4])
                pk = ne.bitcast(U8)[:, W:W + 5 * G8]
                tmp = ne.bitcast(U8)[:, W + 5 * G8:W + 6 * G8]
                h = [hu[:, i::8] for i in range(8)]

                def term(dst, hs, m, s):
                    if s >= 0:
                        nc.vector.tensor_scalar(dst, hs, float(m), float(s),
                                                op0=AL.bitwise_and,
                                                op1=AL.logical_shift_left)
                    else:
                        # mask to 5 valid bits first: bg pixels hold 0xFF
                        nc.vector.tensor_scalar(dst, hs, 31.0, float(-s),
                                                op0=AL.bitwise_and,
                                                op1=AL.logical_shift_right)

                # b0 = h0 | (h1&7)<<5 ; b1 = h1>>3 | (h2&31)<<2 | (h3&1)<<7
                # b2 = h3>>1 | (h4&15)<<4 ; b3 = h4>>4 | (h5&31)<<1 | (h6&3)<<6
                # b4 = h6>>2 | (h7&31)<<3
                specs = [
                    (0, [(0, 31, 0), (1, 7, 5)]),
                    (1, [(1, 0, -3), (2, 31, 2), (3, 1, 7)]),
                    (2, [(3, 0, -1), (4, 15, 4)]),
                    (3, [(4, 0, -4), (5, 31, 1), (6, 3, 6)]),
                    (4, [(6, 0, -2), (7, 31, 3)]),
                ]
                for lane, terms in specs:
                    dst = pk[:, lane::5]
                    hi, m, s = terms[0]
                    term(dst, h[hi], m, s)
                    for hi, m, s in terms[1:]:
                        term(tmp, h[hi], m, s)
                        nc.vector.tensor_tensor(dst, dst, tmp,
                                                op=AL.bitwise_or)
                dpk = self.outs["out"][0:1,
                                       2 * SW + b * 128 * 5 * G8:
                                       2 * SW + (b + 1) * 128 * 5 * G8]
                dpkr = dpk.rearrange("o (p w) -> (o p) w", w=5 * G8)
                for hh in range(2):
                    half = 5 * G8 // 2
                    nc.sync.dma_start(dpkr[:, hh * half:(hh + 1) * half],
                                      pk[:, hh * half:(hh + 1) * half])
            ci = self.S()
            civ = ci.bitcast(I32)[:, :W]
            nc.vector.tensor_copy(civ[:, 0:self.ncyc], chga[:])
            cdst = self.outs["out"][0:1, 2 * SW + 5 * SW // 8:2 * SW + 5 * SW // 8 + 512 * self.ncyc]
            nc.sync.dma_start(
                cdst.rearrange("o (p j) -> (o p) j", j=4 * self.ncyc),
                ci.bitcast(U8)[:, 0:4 * self.ncyc])

    # ------------------------------------------------------------------
    def _build_gates(self, blob_r, zrow8, emask):
        """Unpack fg per tile pair and build L1 gates gh1f/gv1f on device.

        gh[I,J] (J>=1) = BIG*NOT(EH1[I,J-1]); gv[I,J] (I>=1) = BIG*NOT(
        EV1[I-1,J]); col0/row0 = BIG. EH1/EV1 per the reference quadrant
        formulas; EH1 staged in L1R (f32) then converted, same for EV1."""
        nc, tc = self.nc, self.tc
        r1, w1 = LEV[1]
        with tc.tile_pool(name="gates", bufs=1) as pp:
            def unpack(b):
                pk = pp.tile([128, WP], U8, tag="pk", name="pk", bufs=2)
                nc.sync.dma_start(pk[:], blob_r[b])
                g = pp.tile([128, W], U8, tag="g", name="g", bufs=2)
                for j in range(8):
                    nc.vector.tensor_scalar(g[:, j::8], pk[:], float(7 - j),
                                            1.0,
                                            op0=AL.logical_shift_right,
                                            op1=AL.bitwise_and)
                return g

            gz = pp.tile([128, W], U8, tag="g", name="gz", bufs=2)
            nc.vector.memset(gz[0:1, :], 0)
            nc.sync.dma_start(zrow8[:], gz[0:1, :])
            g_next = unpack(0)
            for b in range(YT):
                g = g_next
                nc.vector.tensor_scalar(self.GBb[b][:], g[:], -BIG, BIG,
                                        op0=AL.mult, op1=AL.add)
                g_next = unpack(b + 1) if b + 1 < YT else None
                gup = pp.tile([128, W], U8, tag="gup", name="gup")
                nc.sync.dma_start(gup[0:127, :], g[1:128, :])
                if g_next is not None:
                    nc.sync.dma_start(gup[127:128, :], g_next[0:1, :])
                else:
                    nc.sync.dma_start(gup[127:128, :], zrow8[:])
                # ---- EH1 terms ----
                S = self.S()
                S2 = self.S2()
                # EH0 = g & g>>x
                nc.vector.tensor_tensor(S[:, :W - 1], g[:, :W - 1], g[:, 1:],
                                        op=AL.mult)
                nc.vector.memset(S[:, W - 1:], 0.0)
                # ED1 = g & gup>>x
                nc.vector.tensor_tensor(S2[:, :W - 1], g[:, :W - 1],
                                        gup[:, 1:], op=AL.mult)
                nc.vector.memset(S2[:, W - 1:], 0.0)
                # Podd = max(EH0, ED1*evenrow)
                nc.vector.scalar_tensor_tensor(S[:], S2[:], emask[:, 0:1],
                                               S[:], op0=AL.mult, op1=AL.max)
                # Pu = max(Podd, shiftup(Podd)) (even rows valid)
                nc.sync.dma_start(S2[0:127, :], S[1:128, :])
                nc.vector.tensor_tensor(S[0:127, :], S[0:127, :], S2[0:127, :],
                                        op=AL.max)
                # X[:, J<w1-1] = max(Pu[:, 2J+1], ED2[:, 2J+2]);
                # ED2[:, 2J+2] = g[:, 2J+2] * gup[:, 2J+1]
                nc.vector.tensor_tensor(S2[:, :w1 - 1], g[:, 2:W:2],
                                        gup[:, 1:W - 1:2], op=AL.mult)
                nc.vector.tensor_tensor(S2[:, :w1 - 1], S2[:, :w1 - 1],
                                        S[:, 1:W - 2:2], op=AL.max)
                nc.vector.memset(S2[:, w1 - 1:w1], 0.0)
                # compact even rows -> EH1 stage rows [64b:64b+64)
                nc.sync.dma_start(
                    self.L1R[b // 2][64 * (b % 2):64 * (b % 2) + 64, :],
                    S2[0:128:2, :w1])
            # gh1f = BIG*(1 - EH1 shifted right by one coarse col)
            for i in range(2):
                nc.vector.tensor_scalar(self.gh1f[i][:, 1:],
                                        self.L1R[i][:, :w1 - 1], -BIG, BIG,
                                        op0=AL.mult, op1=AL.add)
                nc.vector.memset(self.gh1f[i][:, 0:1], BIG)
            # ---- EV1 terms (re-unpack, rolling) ----
            g_next = unpack(0)
            for b in range(YT):
                g = g_next
                g_next = unpack(b + 1) if b + 1 < YT else None
                gup = pp.tile([128, W], U8, tag="gup", name="gup")
                nc.sync.dma_start(gup[0:127, :], g[1:128, :])
                if g_next is not None:
                    nc.sync.dma_start(gup[127:128, :], g_next[0:1, :])
                else:
                    nc.sync.dma_start(gup[127:128, :], zrow8[:])
                S = self.S()
                S2 = self.S2()
                # EV0 = g & gup
                nc.vector.tensor_tensor(S[:], g[:], gup[:], op=AL.mult)
                # t1 = max(EV0[ev], ED1[ev]); ED1[2J] = g[2J]*gup[2J+1]
                nc.vector.tensor_tensor(S2[:, :w1], g[:, 0:W:2],
                                        gup[:, 1:W:2], op=AL.mult)
                nc.vector.tensor_tensor(S2[:, :w1], S2[:, :w1], S[:, 0:W:2],
                                        op=AL.max)
                # t2 = max(EV0[odd], ED2[odd]); ED2[2J+1] = g[2J+1]*gup[2J]
                nc.vector.tensor_tensor(S2[:, w1:2 * w1], g[:, 1:W:2],
                                        gup[:, 0:W:2], op=AL.mult)
                nc.vector.tensor_tensor(S2[:, w1:2 * w1], S2[:, w1:2 * w1],
                                        S[:, 1:W:2], op=AL.max)
                # Vfine = max(t1, t2) (odd rows valid)
                nc.vector.tensor_tensor(S2[:, :w1], S2[:, :w1],
                                        S2[:, w1:2 * w1], op=AL.max)
                # compact odd rows, shifted down one coarse row:
                # gv row (64b + q + 1) <- Vfine fine row 2q+1
                lo = 64 * b + 1
                hi = min(64 * b + 65, r1)
                n1 = min(hi, 128) - lo if lo < 128 else 0
                if n1 > 0:
                    nc.sync.dma_start(self.L1R[0][lo:lo + n1, :],
                                      S2[1:2 * n1:2, :w1])
                if hi > 128:
                    lo2 = max(lo, 128)
                    q0 = lo2 - (64 * b + 1)
                    n2 = hi - lo2
                    nc.sync.dma_start(
                        self.L1R[1][lo2 - 128:lo2 - 128 + n2, :],
                        S2[2 * q0 + 1:2 * (q0 + n2):2, :w1])
            # gv1f = BIG*(1 - stage); row0 = BIG
            for i in range(2):
                nc.vector.tensor_scalar(self.gv1f[i][:, :], self.L1R[i][:, :],
                                        -BIG, BIG, op0=AL.mult, op1=AL.add)
            nc.vector.memset(self.gv1f[0][0:1, :], BIG)

    # ------------------------------------------------------------------
    def l0_rep(self):
        """Jacobi 3x3 masked min sweep + fwd/bwd masked row scans."""
        nc = self.nc
        R0, GBb = self.R0, self.GBb
        rs = self.rowsnap
        for b in range(YT):
            nc.sync.dma_start(rs[2 * b:2 * b + 1, :], R0[b][0:1, :])
            nc.sync.dma_start(rs[2 * b + 1:2 * b + 2, :], R0[b][127:128, :])
        for b in range(YT):
            S = self.S()
            nc.sync.dma_start(S[0:127, :], R0[b][1:128, :])
            if b + 1 < YT:
                nc.sync.dma_start(S[127:128, :],
                                  rs[2 * (b + 1):2 * (b + 1) + 1, :])
            else:
                nc.sync.dma_start(S[127:128, :], self.bigrow[:])
            nc.vector.tensor_tensor(S[:], R0[b][:], S[:], op=AL.min)
            S2 = self.S2()
            nc.sync.dma_start(S2[1:128, :], R0[b][0:127, :])
            if b > 0:
                nc.sync.dma_start(S2[0:1, :],
                                  rs[2 * (b - 1) + 1:2 * (b - 1) + 2, :])
            else:
                nc.sync.dma_start(S2[0:1, :], self.bigrow[:])
            nc.vector.tensor_tensor(S[:], S[:], S2[:], op=AL.min)
            nc.vector.tensor_tensor(R0[b][:, 1:], S[:, 1:], S[:, :-1],
                                    op=AL.min)
            nc.vector.tensor_copy(R0[b][:, 0:1], S[:, 0:1])
            nc.vector.tensor_tensor(R0[b][:, :-1], R0[b][:, :-1], S[:, 1:],
                                    op=AL.min)
            nc.vector.tensor_tensor(R0[b][:], R0[b][:], GBb[b][:], op=AL.max)
            nc.vector.tensor_tensor_scan(R0[b][:], GBb[b][:], R0[b][:], BIG,
                                         op0=AL.max, op1=AL.min)
            nc.vector.tensor_tensor_scan(R0[b][:, ::-1], GBb[b][:, ::-1],
                                         R0[b][:, ::-1], BIG,
                                         op0=AL.max, op1=AL.min)

    def lev_smooth(self, k):
        """One rep: seg row scans fwd+bwd, then gated +-1 vertical."""
        nc = self.nc
        rk, wk = LEV[k]
        Rt = self.lab[k]
        pR = min(rk, 128)
        n = len(Rt)
        gh = self.gh1f if k == 1 else self.Hf[k]
        gv = self.gv1f if k == 1 else self.Vf[k]
        for i in range(n):
            nc.vector.tensor_tensor_scan(Rt[i][:, :], gh[i][:, :], Rt[i][:, :],
                                         BIG, op0=AL.max, op1=AL.min)
            nc.vector.tensor_tensor_scan(
                Rt[i][:, wk - 2::-1], gh[i][:, wk - 1:0:-1],
                Rt[i][:, wk - 2::-1], Rt[i][:, wk - 1:wk],
                op0=AL.max, op1=AL.min)
        S = self.S()
        for i in range(n):
            sl = S[:pR, i * wk:(i + 1) * wk]
            nc.sync.dma_start(sl[1:pR, :], Rt[i][0:pR - 1, :])
            if i > 0:
                nc.sync.dma_start(sl[0:1, :], Rt[i - 1][pR - 1:pR, :])
            else:
                nc.sync.dma_start(sl[0:1, :], self.bigrow[:, :wk])
        for i in range(n):
            sl = S[:pR, i * wk:(i + 1) * wk]
            nc.vector.tensor_tensor(sl, sl, gv[i][:, :], op=AL.add)
            nc.vector.tensor_tensor(Rt[i][:, :], Rt[i][:, :], sl, op=AL.min)
        S2 = self.S2()
        for i in range(n):
            u = S2[:pR, i * wk:(i + 1) * wk]
            nc.vector.tensor_tensor(u, Rt[i][:, :], gv[i][:, :], op=AL.add)
        S = self.S()
        for i in range(n):
            sl = S[:pR, i * wk:(i + 1) * wk]
            nc.sync.dma_start(sl[0:pR - 1, :], S2[1:pR, i * wk:(i + 1) * wk])
            if i + 1 < n:
                nc.sync.dma_start(sl[pR - 1:pR, :],
                                  S2[0:1, (i + 1) * wk:(i + 2) * wk])
            else:
                nc.sync.dma_start(sl[pR - 1:pR, :], self.bigrow[:, :wk])
            nc.vector.tensor_tensor(Rt[i][:, :], Rt[i][:, :], sl, op=AL.min)

    def restrict01(self):
        nc = self.nc
        r1, w1 = LEV[1]
        for b in range(YT):
            S2 = self.S2()
            nc.vector.tensor_tensor(S2[:, :w1], self.R0[b][:, 0:W:2],
                                    self.R0[b][:, 1:W:2], op=AL.min)
            S = self.S()
            nc.sync.dma_start(S[0:127, :w1], S2[1:128, :w1])
            nc.vector.tensor_tensor(S2[:, :w1], S2[:, :w1], S[:, :w1],
                                    op=AL.min)
            nc.sync.dma_start(
                self.L1R[b // 2][64 * (b % 2):64 * (b % 2) + 64, :],
                S2[0:128:2, :w1])

    def deep_down(self, k):
        nc = self.nc
        rf, wf = LEV[k - 1]
        rk, wk = LEV[k]
        pRf = min(rf, 128)
        nRf = (rf + 127) // 128
        pRfh = pRf // 2
        Rf = self.lab[k - 1]
        Mk = self.lab[k][0]
        ghf = self.gh1f if k == 2 else self.Hf[k - 1]
        gvf = self.gv1f if k == 2 else self.Vf[k - 1]
        # restrict
        for i in range(nRf):
            S2 = self.S2()
            nc.vector.tensor_tensor(S2[:pRf, :wk], Rf[i][:, 0:wf:2],
                                    Rf[i][:, 1:wf:2], op=AL.min)
            S = self.S()
            nc.sync.dma_start(S[0:pRf - 1, :wk], S2[1:pRf, :wk])
            nc.vector.tensor_tensor(S2[:pRf, :wk], S2[:pRf, :wk],
                                    S[:pRf, :wk], op=AL.min)
            nc.sync.dma_start(Mk[i * pRfh:(i + 1) * pRfh, :],
                              S2[0:pRf:2, :wk])
        nc.vector.tensor_copy(self.snapM[k][:, :], Mk[:, :])
        # gates
        S2g = self.S2()
        for i in range(nRf):
            S = self.S()
            up = S[:pRf, 2 * wk:3 * wk]
            nc.sync.dma_start(up[0:pRf:2, :],
                              self.snapM[k][i * pRfh:(i + 1) * pRfh, :])
            nc.sync.dma_start(up[1:pRf:2, :],
                              self.snapM[k][i * pRfh:(i + 1) * pRfh, :])
            nef = (S2g[:pRf, i * wf:(i + 1) * wf] if nRf > 1
                   else S2g[:pRf, :wf])
            nc.vector.tensor_tensor(nef, Rf[i][:, :], dbl(up),
                                    op=AL.not_equal)
            # Hf: Aev[:, J] = gf[:, 2J] + BIG*(nef[:, 2J-1] + nef[:, 2J])
            Aev = S[:pRf, 0:wk]
            Sv = S[:pRf, wk:2 * wk]
            nc.vector.tensor_tensor(Aev[:, 1:wk], nef[:, 1:wf - 2:2],
                                    nef[:, 2:wf:2], op=AL.add)
            nc.vector.scalar_tensor_tensor(Aev[:, 1:wk], Aev[:, 1:wk], BIG,
                                           ghf[i][:, 2:wf:2],
                                           op0=AL.mult, op1=AL.add)
            nc.vector.memset(Aev[:, 0:1], BIG)
            nc.sync.dma_start(Sv[0:pRf - 1, :], Aev[1:pRf, :])
            nc.vector.tensor_tensor(Aev, Aev, Sv, op=AL.min)
            nc.sync.dma_start(self.Hf[k][0][i * pRfh:(i + 1) * pRfh, :],
                              Aev[0:pRf:2, :])
        # Vf: B = gvf + BIG*(nef_up + nef); x-pair min; row compact
        for i in range(nRf):
            nef = (S2g[:pRf, i * wf:(i + 1) * wf] if nRf > 1
                   else S2g[:pRf, :wf])
            S = self.S()
            Sv = S[:pRf, 0:wf]
            nc.sync.dma_start(Sv[1:pRf, :], nef[0:pRf - 1, :])
            if i > 0:
                pnef = S2g[:pRf, (i - 1) * wf:i * wf]
                nc.sync.dma_start(Sv[0:1, :], pnef[pRf - 1:pRf, :])
            nc.vector.tensor_tensor(Sv, Sv, nef, op=AL.add)
            nc.vector.scalar_tensor_tensor(Sv, Sv, BIG, gvf[i][:, :],
                                           op0=AL.mult, op1=AL.add)
            Bp = S[:pRf, wf:wf + wk]
            nc.vector.tensor_tensor(Bp, Sv[:, 0:wf:2], Sv[:, 1:wf:2],
                                    op=AL.min)
            nc.sync.dma_start(self.Vf[k][0][i * pRfh:(i + 1) * pRfh, :],
                              Bp[0:pRf:2, :])
        nc.vector.memset(self.Vf[k][0][0:1, :], BIG)
        for _ in range(NS1):
            self.lev_smooth(k)

    def deep_up(self, k):
        nc = self.nc
        rf, wf = LEV[k - 1]
        rk, wk = LEV[k]
        pRf = min(rf, 128)
        nRf = (rf + 127) // 128
        pRfh = pRf // 2
        Rf = self.lab[k - 1]
        Mk = self.lab[k][0]
        for i in range(nRf):
            S = self.S()
            upl = S[:pRf, 0:wk]
            ups = S[:pRf, wk:2 * wk]
            nc.sync.dma_start(upl[0:pRf:2, :],
                              Mk[i * pRfh:(i + 1) * pRfh, :])
            nc.sync.dma_start(upl[1:pRf:2, :],
                              Mk[i * pRfh:(i + 1) * pRfh, :])
            nc.sync.dma_start(ups[0:pRf:2, :],
                              self.snapM[k][i * pRfh:(i + 1) * pRfh, :])
            nc.sync.dma_start(ups[1:pRf:2, :],
                              self.snapM[k][i * pRfh:(i + 1) * pRfh, :])
            ne = self.S2()
            nc.vector.tensor_tensor(ne[:pRf, :wf], Rf[i][:, :], dbl(ups),
                                    op=AL.not_equal)
            nc.vector.scalar_tensor_tensor(ne[:pRf, :wf], ne[:pRf, :wf], BIG,
                                           dbl(upl), op0=AL.mult, op1=AL.add)
            nc.vector.tensor_tensor(Rf[i][:, :], Rf[i][:, :], ne[:pRf, :wf],
                                    op=AL.min)
        if k - 1 >= 2:
            for _ in range(NS1):
                self.lev_smooth(k - 1)

    def prolong10(self):
        nc = self.nc
        r1, w1 = LEV[1]
        for b in range(YT):
            up = self.S()
            src = self.L1R[b // 2][64 * (b % 2):64 * (b % 2) + 64, :]
            nc.sync.dma_start(up[0:128:2, :w1], src)
            nc.sync.dma_start(up[1:128:2, :w1], src)
            nc.vector.tensor_tensor(self.R0[b][:], self.R0[b][:],
                                    dbl(up[:, :w1]), op=AL.min)
            nc.vector.tensor_tensor(self.R0[b][:], self.R0[b][:],
                                    self.GBb[b][:], op=AL.max)

    def cycle(self, c, chga):
        nc = self.nc
        for _ in range(NS0):
            self.l0_rep()
        self.restrict01()
        for _ in range(NS1):
            self.lev_smooth(1)
        for k in range(2, NLEV):
            self.deep_down(k)
        for k in range(NLEV - 1, 1, -1):
            self.deep_up(k)
        self.prolong10()
        self.l0_rep()
        if c < self.ncyc - 1:
            # change tracking only needed for the certificate (final cycle);
            # refresh prevt just before it so the comparison is right
            if c == self.ncyc - 2:
                for b in range(YT):
                    nc.sync.dma_start(self.prevt[b][:], self.R0[b][:])
            return
        for b in range(YT):
            pv = self.S()
            nc.sync.dma_start(pv[:], self.prevt[b][:])
            ne = self.S2()
            nc.vector.tensor_tensor(ne[:], self.R0[b][:], pv[:],
                                    op=AL.not_equal)
            red = self.scr.tile([128, 1], F32, tag="red", name="red")
            nc.vector.tensor_reduce(red[:], ne[:], axis=AX.X, op=AL.add)
            nc.vector.tensor_tensor(chga[:, c:c + 1], chga[:, c:c + 1],
                                    red[:], op=AL.add)
            nc.sync.dma_start(self.prevt[b][:], self.R0[b][:])


def build_program(ncyc=NCYC):
    nc = bacc.Bacc("TRN2", target_bir_lowering=False, debug=False,
                   num_devices=NCORES)
    ins = {
        "blob": nc.dram_tensor("blob", [SR, WP], U8,
                               kind="ExternalInput").ap(),
    }
    outs = {
        "out": nc.dram_tensor("out", [1, 2 * SR * W + 5 * SR * W // 8 + 512 * ncyc], U8,
                              kind="ExternalOutput").ap(),
    }
    with tile.TileContext(nc) as tc:
        Dev(tc, ins, outs, ncyc).build()
    nc.compile()
    return nc


# ---------------------------------------------------------------------------
# host side
# ---------------------------------------------------------------------------

def make_blob(f):
    return np.packbits(f, axis=1)


def decode_out(o):
    """o: u8 flat device output -> (e = lab-1 raw int32 [SR, W] (bg garbage,
    caller masks by fg), chg [128, NCYC])."""
    SW = SR * W
    p01 = o[:2 * SW].view(np.uint16).reshape(SR, W)
    pkb = o[2 * SW:2 * SW + 5 * SW // 8].reshape(-1, 5).astype(np.int32)
    b0, b1, b2, b3, b4 = (pkb[:, i] for i in range(5))
    h = np.empty((pkb.shape[0], 8), np.int32)
    h[:, 0] = b0 & 31
    h[:, 1] = (b0 >> 5) | ((b1 & 3) << 3)
    h[:, 2] = (b1 >> 2) & 31
    h[:, 3] = ((b1 >> 7) & 1) | ((b2 & 15) << 1)
    h[:, 4] = (b2 >> 4) | ((b3 & 1) << 4)
    h[:, 5] = (b3 >> 1) & 31
    h[:, 6] = ((b3 >> 6) & 3) | ((b4 & 7) << 2)
    h[:, 7] = b4 >> 3
    e = p01.astype(np.int32) | (h.reshape(SR, W) << 16)
    co = 2 * SW + 5 * SW // 8
    chg = o[co:co + 512 * NCYC].view(np.int32).reshape(128, NCYC)
    return e, chg


def seam_classes(labs):
    """labs: per-strip LOCAL label arrays. Union-find over seam equivalences.
    Returns (cu, cv): sorted GLOBAL labels that change -> new global value."""
    pairs_a, pairs_b = [], []
    for c in range(NCORES - 1):
        A = labs[c][SR - 1].astype(np.int64)
        B = labs[c + 1][0].astype(np.int64)
        A = np.where(A > 0, A + c * SR * W, 0)
        B = np.where(B > 0, B + (c + 1) * SR * W, 0)
        for sh in (-1, 0, 1):
            Bs = np.roll(B, sh)
            valid = (A > 0) & (Bs > 0)
            if sh == 1:
                valid[0] = False
            if sh == -1:
                valid[-1] = False
            pairs_a.append(A[valid])
            pairs_b.append(Bs[valid])
    ea = np.concatenate(pairs_a)
    eb = np.concatenate(pairs_b)
    if len(ea) == 0:
        return np.empty(0, np.int64), np.empty(0, np.int32)
    u = np.unique(np.concatenate([ea, eb]))
    ia = np.searchsorted(u, ea)
    ib = np.searchsorted(u, eb)
    val = u.copy()
    for _ in range(100):
        old = val.copy()
        nv = val.copy()
        np.minimum.at(nv, ia, val[ib])
        np.minimum.at(nv, ib, val[ia])
        nv = np.minimum(nv, nv[np.searchsorted(u, nv)])
        val = nv
        if np.array_equal(val, old):
            break
    ch = val != u
    return u[ch], val[ch].astype(np.int32)


# -- numpy multigrid fallback (never triggers when the device converges) --

_K64 = np.int64(2 ** 26)


def _seg_scan(X, G, axis, reverse=False):
    if reverse:
        X = np.flip(X, axis=axis); G = np.flip(G, axis=axis)
    brk = G >= BIGI
    seg = np.cumsum(brk, axis=axis).astype(np.int64)
    sp = (X.shape[axis] + 2) - seg
    C = np.minimum.accumulate(X + sp * _K64, axis=axis)
    res = np.minimum(C - sp * _K64, X)
    if reverse:
        res = np.flip(res, axis=axis)
    return res


def _sweep3(lab, fg):
    h, w = lab.shape
    p = np.full((h + 2, w + 2), BIGI)
    p[1:-1, 1:-1] = lab
    m = lab.copy()
    for di in range(3):
        for dj in range(3):
            m = np.minimum(m, p[di:di + h, dj:dj + w])
    return np.where(fg, m, BIGI)


def _lscan(T, gh, gv):
    Hb = np.full(gh.shape, BIGI); Hb[:, :-1] = gh[:, 1:]
    Vb = np.full(gv.shape, BIGI); Vb[:-1, :] = gv[1:, :]
    T = _seg_scan(T, gh, 1)
    T = _seg_scan(T, Hb, 1, reverse=True)
    T = _seg_scan(T, gv, 0)
    return _seg_scan(T, Vb, 0, reverse=True)


def _restr(lab):
    return np.minimum(np.minimum(lab[0::2, 0::2], lab[0::2, 1::2]),
                      np.minimum(lab[1::2, 0::2], lab[1::2, 1::2]))


def _np_gates_l1(f):
    def q(A, i, j):
        return A[i::2, j::2]
    EH0 = f & np.roll(f, -1, 1); EH0[:, -1] = False
    EV0 = f & np.roll(f, -1, 0); EV0[-1, :] = False
    ED1 = f & np.roll(np.roll(f, -1, 0), -1, 1)
    ED1[-1, :] = False; ED1[:, -1] = False
    ED2 = f & np.roll(np.roll(f, -1, 0), 1, 1)
    ED2[-1, :] = False; ED2[:, 0] = False
    EH1 = (q(EH0, 0, 1) | q(EH0, 1, 1) | q(ED1, 0, 1)
           | q(np.roll(ED2, -2, 1), 0, 0))
    EH1[:, -1] = False
    EV1 = q(EV0, 1, 0) | q(EV0, 1, 1) | q(ED1, 1, 0) | q(ED2, 1, 1)
    EV1[-1, :] = False
    s2, w2 = f.shape[0] // 2, f.shape[1] // 2
    gh = np.full((s2, w2), BIGI); gh[:, 1:] = np.where(EH1[:, :-1], 0, BIGI)
    gv = np.full((s2, w2), BIGI); gv[1:, :] = np.where(EV1[:-1, :], 0, BIGI)
    return gh, gv


def _np_coarse_gates(gh, gv, nef):
    shp = (gh.shape[0] // 2, gh.shape[1] // 2)
    Hf = np.full(shp, BIGI)
    t1 = gh[0::2, 0::2] + (np.roll(nef[0::2, 1::2], 1, 1) + nef[0::2, 0::2]) * BIGI
    t2 = gh[1::2, 0::2] + (np.roll(nef[1::2, 1::2], 1, 1) + nef[1::2, 0::2]) * BIGI
    Hf[:, 1:] = np.minimum(t1, t2)[:, 1:]
    Vf = np.full(shp, BIGI)
    t1 = gv[0::2, 0::2] + (np.roll(nef[1::2, 0::2], 1, 0) + nef[0::2, 0::2]) * BIGI
    t2 = gv[0::2, 1::2] + (np.roll(nef[1::2, 1::2], 1, 0) + nef[0::2, 1::2]) * BIGI
    Vf[1:, :] = np.minimum(t1, t2)[1:, :]
    return Hf, Vf


def _host_finish_local(lab, fg):
    """Warm-start numpy multigrid (full column scans) to the strip fixpoint.
    Device labels are sound (monotone upper bounds whose values are member
    indices), so continuing from them is valid; the loop ends on a no-change
    cycle, whose leading 3x3 sweep certifies exactness."""
    lab = np.where(fg & (lab > 0), lab.astype(np.int64), BIGI)
    gadd = np.where(fg, 0, BIGI).astype(np.int64)
    gh1, gv1 = _np_gates_l1(fg)
    for _ in range(200):
        new = _sweep3(lab, fg)
        new = _seg_scan(new, gadd, 1)
        new = _seg_scan(new, gadd, 1, reverse=True)
        labs = {1: _restr(new)}
        gh = {1: gh1}; gv = {1: gv1}
        snaps = {}
        for _r in range(2):
            labs[1] = _lscan(labs[1], gh1, gv1)
        for k in range(2, 6):
            fine = labs[k - 1]
            Lmin = _restr(fine)
            snaps[k] = Lmin
            nef = (fine != np.repeat(np.repeat(Lmin, 2, 0), 2, 1)).astype(np.int64)
            gh[k], gv[k] = _np_coarse_gates(gh[k - 1], gv[k - 1], nef)
            cur = Lmin.copy()
            for _r in range(2):
                cur = _lscan(cur, gh[k], gv[k])
            labs[k] = cur
        for k in range(5, 1, -1):
            up = np.repeat(np.repeat(labs[k], 2, 0), 2, 1)
            upm = np.repeat(np.repeat(snaps[k], 2, 0), 2, 1)
            labs[k - 1] = np.minimum(labs[k - 1],
                                     up + (labs[k - 1] != upm) * BIGI)
            if k - 1 >= 2:
                for _r in range(2):
                    labs[k - 1] = _lscan(labs[k - 1], gh[k - 1], gv[k - 1])
        up = np.repeat(np.repeat(labs[1], 2, 0), 2, 1)
        new = np.where(fg, np.minimum(new, up), BIGI)
        new = _sweep3(new, fg)
        new = _seg_scan(new, gadd, 1)
        new = _seg_scan(new, gadd, 1, reverse=True)
        if np.array_equal(new, lab):
            break
        lab = new
    return np.where(lab >= BIGI, 0, lab).astype(np.int32)


_CACHED = {}


def kernel(prob):
    prob2 = np.squeeze(np.asarray(prob))
    fg = prob2 > 0.5

    if 'nc' not in _CACHED:
        _CACHED['nc'] = build_program()
    nc = _CACHED['nc']

    in_maps = [{"blob": make_blob(fg[c * SR:(c + 1) * SR])}
               for c in range(NCORES)]
    res = run_bass_kernel_spmd(nc, in_maps, core_ids=list(range(NCORES)))
    kernel._launches = 1
    labs = []
    chgs = []
    for c in range(NCORES):
        e, chg = decode_out(res.results[c]["out"][0])
        f = fg[c * SR:(c + 1) * SR]
        labs.append(np.where(f, e + 1, 0).astype(np.int32))
        chgs.append(chg.sum(axis=0))
    kernel._chgs = np.stack(chgs)
    for c in range(NCORES):
        if kernel._chgs[c, -1] != 0:
            # fallback: finish this strip on host (local labels stay local)
            f = fg[c * SR:(c + 1) * SR]
            labs[c] = _host_finish_local(labs[c], f)
    cu, cv = seam_classes(labs)
    # per-strip local LUT (strip-local labels <= SR*W) -> one take per strip
    full = np.empty((H, W), np.int32)
    for c in range(NCORES):
        off = np.int64(c * SR * W)
        lut = np.arange(off, off + SR * W + 1, dtype=np.int32)
        lut[0] = 0
        if len(cu):
            m = (cu > off) & (cu <= off + SR * W)
            lut[(cu[m] - off).astype(np.int64)] = cv[m]
        full[c * SR:(c + 1) * SR] = lut.take(labs[c])
    return full


if __name__ == '__main__':
    import time
    mode = sys.argv[1] if len(sys.argv) > 1 else 'e2e'
    fgA = np.load('/root/problem/work/fg.npy')
    if mode == 'parity':
        ncyc = int(sys.argv[2]) if len(sys.argv) > 2 else 1
        ncb = build_program(ncyc)
        in_maps = [{"blob": make_blob(fgA[c * SR:(c + 1) * SR])}
                   for c in range(NCORES)]
        t0 = time.time()
        res = run_bass_kernel_spmd(ncb, in_maps, core_ids=list(range(NCORES)))
        print(f"launch: {time.time() - t0:.1f}s", flush=True)
        sys.path.insert(0, '/root/problem')
        from model import cycle as mcycle, make_l1_gates
        from model import BIGI as MBIGI
        for c in range(NCORES):
            f = fgA[c * SR:(c + 1) * SR]
            idx = np.arange(SR * W, dtype=np.int64).reshape(SR, W) + 1
            mlab = np.where(f, idx, MBIGI)
            gadd = np.where(f, 0, MBIGI).astype(np.int64)
            gh1, gv1 = make_l1_gates(f)
            mchg = []
            for _ in range(ncyc):
                new = mcycle(mlab, f, gadd, gh1, gv1, l0col=False, nlev=6,
                             vml=True, ns0=NS0, ns1=NS1)
                mchg.append(int((new != mlab).sum()))
                mlab = new
            mres = np.where(mlab >= MBIGI, 0, mlab).astype(np.int32)
            oo = res.results[c]["out"][0]
            globals()['NCYC'] = ncyc
            e, dch = decode_out(oo)
            got = np.where(f, e + 1, 0).astype(np.int32)
            dchg = dch.sum(axis=0).astype(np.int64)
            eq = np.array_equal(got, mres)
            print(f"core {c}: equal={eq} dev_chg={list(dchg)} model_chg={mchg}")
            if not eq:
                bad = np.argwhere(got != mres)
                print(f"  nbad={len(bad)} first={bad[:5].tolist()}")
                for y, x in bad[:5]:
                    print(f"   ({y},{x}) got={got[y, x]} exp={mres[y, x]} "
                          f"fg={f[y, x]}")
                break
    else:
        prob = np.where(fgA, 0.9, 0.1).astype(np.float32)[None, None]
        exp = np.load('/root/problem/work/ref_out.npy')
        t0 = time.time()
        out = kernel(prob)
        print(f"kernel cold: {time.time() - t0:.2f}s", flush=True)
        print("equal:", np.array_equal(out, exp))
        print("chg last2:", kernel._chgs[:, -2:].astype(int).tolist())
        for _ in range(2):
            t0 = time.time()
            out = kernel(prob)
            print(f"kernel warm: {time.time() - t0:.2f}s "
                  f"equal={np.array_equal(out, exp)}")
